# revision 27
# baseline (speedup 1.0000x reference)
"""Two-layer GCN encoder on 8 Trainium2 NeuronCores.

Strategy (dst-partitioned, matmul-based segment sum, fp16 internal):
  - Nodes are grouped into 392 blocks of 128; blocks are assigned to
    (core, slot) pairs balancing edge counts, 49 slots per core.
  - Every edge is owned by the core owning its dst block, so each core's
    aggregation for its blocks is complete: no all-reduce needed.
  - Per edge tile (128 edges): get the src rows into SBUF partitions
    (layer 1: host-pregathered xg1 slab loads; layer 2: dma_gather from
    the h pieces), and accumulate aggT[feat, node] += Xg.T @ P in PSUM,
    where P[e, n] = (dstcol==n)*w is HOST-PREBUILT (pmt input, shared by
    both layers) so no on-device P construction is needed.
  - Per block: h = relu(aggT.T @ W + b) via two matmuls (bias as a K=1
    matmul) and an ACT relu eviction (fp16 for layer 1, fp32 output for
    layer 2).

Overlap structure: slots are split at 25 into parts A/B.  h is
all-gathered in TWO pieces (AG_A after layer-1 slots 0-24, AG_B at the
end of layer 1).  Edges are partitioned by their SRC slot's part; layer 2
runs part-A gathers (reading h_fullA) concurrently with the tail of
layer-1 compute, then part-B.  Each piece's row space (25600 / 24576
rows) fits dma_gather's int16 indices, replacing the old lo/hi split.
Per-dst-slot aggregates accumulate part A into an fp16 partial, combined
with part B in a DVE add before the dense W matmul.

dma_gather is limited to 1024 indices/call (ring capacity: the device
hard-crashes above it) and costs ~8us/call of serial GpSimd time, which
is the main remaining bottleneck.
"""

import os

import numpy as np
from concourse import bacc, bass, mybir, tile
from concourse.bass_utils import run_bass_kernel_spmd

P = 128
N_NODES = 50000
N_EDGES = 800000
NFEAT = 128
NC = 8
SLOTS = 49                 # node blocks per core
NB = NC * SLOTS            # 392 blocks, 50176 padded rows
SHARD = SLOTS * P          # 6272 rows per core
NFULL = NB * P             # 50176
ASLOTS = 25                # slots in part A (rest in part B)
BSLOTS = SLOTS - ASLOTS
SHA = ASLOTS * P           # 3200 rows per core in piece A
SHB = BSLOTS * P           # 3072 rows per core in piece B
NFA = NC * SHA             # 25600 (< 32768: fits int16 gather idx)
NFB = NC * SHB             # 24576
GROUP = 5                  # dst slots per gather group
CALL_TILES = 8             # dma_gather device-crashes above 1024 idxs/call

SINGLE_PACKET = bool(int(os.environ.get("GCN_SP", "0")))

FP32 = mybir.dt.float32
FP16 = mybir.dt.float16

# Set by kernel() for test harness introspection (trace results etc.)
last_run_results = None


def _wrap16(flat):
    """dma_gather index layout: logical i -> [i % 16, i // 16], x8 replicated."""
    n16 = len(flat) // 16
    arr = np.asarray(flat, dtype=np.int16).reshape(n16, 16).T  # [16, n16]
    return np.tile(arr, (8, 1))  # [128, n16]


def _prep(edge_index, edge_weight):
    """Host-side sharding: block assignment, piece gather indices, pm tiles."""
    src = edge_index[0].astype(np.int64)
    dst = edge_index[1].astype(np.int64)
    w = edge_weight.astype(np.float32)

    blk = dst >> 7
    col = (dst & 127).astype(np.float32)

    cnt = np.bincount(blk, minlength=NB)
    order = np.argsort(-cnt, kind="stable")
    # Refine within slabs of 4 slots: re-sort by A-src edge count so each
    # slot's 8 blocks have similar A/B splits (reduces the shared
    # max-over-cores tile schedule).  Partness depends on the assignment,
    # so approximate with a first-pass assignment by total count.
    slot_of0 = np.empty(NB, np.int64)
    ba0 = order.reshape(SLOTS, NC).T
    for c0 in range(NC):
        for s0 in range(SLOTS):
            slot_of0[ba0[c0, s0]] = s0
    sblk0 = src >> 7
    a0 = slot_of0[sblk0] < ASLOTS
    a_cnt = np.bincount(blk[a0], minlength=NB)
    order2 = order.copy()
    for a in range(0, NB, 4 * NC):
        slab = order2[a:a + 4 * NC]
        order2[a:a + 4 * NC] = slab[np.argsort(-a_cnt[slab], kind="stable")]
    block_at = order2.reshape(SLOTS, NC).T          # [core, slot] -> block
    core_of = np.empty(NB, np.int64)
    slot_of = np.empty(NB, np.int64)
    for c in range(NC):
        for s in range(SLOTS):
            core_of[block_at[c, s]] = c
            slot_of[block_at[c, s]] = s

    eorder = np.argsort(blk, kind="stable")
    estart = np.zeros(NB + 1, np.int64)
    np.cumsum(cnt, out=estart[1:])

    # per-edge src part + piece-space gather row
    sblk = src >> 7
    s_core = core_of[sblk]
    s_slot = slot_of[sblk]
    is_a = s_slot < ASLOTS
    vq = np.where(
        is_a,
        s_core * SHA + s_slot * P + (src & 127),
        s_core * SHB + (s_slot - ASLOTS) * P + (src & 127),
    )

    groups = [list(range(g, min(g + GROUP, SLOTS))) for g in range(0, SLOTS, GROUP)]
    NG = len(groups)

    # per (core, slot): A/B edge id lists + shared tile schedule
    ids_cs = [[None] * SLOTS for _ in range(NC)]
    TA = np.zeros(SLOTS, np.int64)
    TB = np.zeros(SLOTS, np.int64)
    for c in range(NC):
        for s in range(SLOTS):
            b = block_at[c, s]
            ids = eorder[estart[b]:estart[b + 1]]
            m = is_a[ids]
            ea, eb = ids[m], ids[~m]
            ids_cs[c][s] = (ea, eb)
            TA[s] = max(TA[s], (len(ea) + P - 1) // P)
            TB[s] = max(TB[s], (len(eb) + P - 1) // P)

    # Tile enumeration: part-major: for q in (A, B): for g: for s in g.
    T_of = [TA, TB]
    seg = []          # [q][g] -> (tid0, ntiles_in_segment)
    tid0 = 0
    for q in range(2):
        segq = []
        for g in groups:
            n = int(sum(T_of[q][s] for s in g))
            segq.append((tid0, n))
            tid0 += n
        seg.append(segq)
    ntiles = tid0

    # gather calls for layer 2: per (q, g) segment, windows of <=8 tiles
    calls = []  # (q, gi, t0_in_seg, ntiles)
    for q in range(2):
        for gi in range(NG):
            _, segn = seg[q][gi]
            t0 = 0
            while t0 < segn:
                nt = min(CALL_TILES, segn - t0)
                calls.append((q, gi, t0, nt))
                t0 += nt

    sched = {"TA": TA, "TB": TB, "groups": groups, "seg": seg,
             "ntiles": ntiles, "calls": calls}

    idx_np = []
    pmt_np = []
    rows_np = []
    ncols = np.arange(P, dtype=np.float32)
    for c in range(NC):
        flat_idx = []
        flat_rows = []
        pmt = np.zeros((P, ntiles * P), np.float16)
        tid = 0
        for q in range(2):
            for g in groups:
                for s in g:
                    ea, eb = ids_cs[c][s]
                    ids = ea if q == 0 else eb
                    n = int(T_of[q][s]) * P
                    iv = np.zeros(n, np.int64)
                    rv = np.zeros(n, np.int64)
                    cv = np.full(n, -1.0, np.float32)
                    wv = np.zeros(n, np.float32)
                    iv[:len(ids)] = vq[ids]
                    rv[:len(ids)] = src[ids]
                    cv[:len(ids)] = col[ids]
                    wv[:len(ids)] = w[ids]
                    flat_idx.append(iv)
                    flat_rows.append(rv)
                    for t in range(int(T_of[q][s])):
                        pm = (cv[t * P:(t + 1) * P, None] == ncols[None, :])
                        pm = pm * wv[t * P:(t + 1) * P, None]
                        pmt[:, tid * P:(tid + 1) * P] = pm.astype(np.float16)
                        tid += 1
        idx_np.append(_wrap16(np.concatenate(flat_idx)))
        rows_np.append(np.concatenate(flat_rows))
        pmt_np.append(pmt)

    return block_at, sched, idx_np, pmt_np, rows_np


def _build(sched, n16):
    """Build the SPMD bass program. Returns finalized nc."""
    nc = bacc.Bacc(num_devices=NC)

    ntiles = sched["ntiles"]
    # Layer-1 gather done on the HOST: xg1[p, t*128 + f] = x16[src[t*128+p], f]
    # in exactly the SBUF layout dma_gather would produce.
    xg1_in = nc.declare_dram_parameter("xg1", [P, ntiles * P], FP16, isOutput=False)
    # host-built P tiles: pmt[e, tid*128+n] = (dstcol==n)*w, both layers
    pmt_in = nc.declare_dram_parameter("pmt", [P, ntiles * P], FP16, isOutput=False)
    w1_in = nc.declare_dram_parameter("W1", [NFEAT, NFEAT], FP32, isOutput=False)
    w2_in = nc.declare_dram_parameter("W2", [NFEAT, NFEAT], FP32, isOutput=False)
    b1_in = nc.declare_dram_parameter("b1", [1, NFEAT], FP32, isOutput=False)
    b2_in = nc.declare_dram_parameter("b2", [1, NFEAT], FP32, isOutput=False)
    idx_in = nc.declare_dram_parameter("idx", [P, n16], mybir.dt.int16, isOutput=False)
    out = nc.declare_dram_parameter("out", [SHARD, NFEAT], FP32, isOutput=True)

    relu = mybir.ActivationFunctionType.Relu
    TA, TB = sched["TA"], sched["TB"]
    T_of = [TA, TB]
    groups = sched["groups"]
    seg = sched["seg"]

    with tile.TileContext(nc) as tc:
        with tc.tile_pool(name="const", bufs=1) as cpool, \
             tc.tile_pool(name="gbuf", bufs=4) as gpool, \
             tc.tile_pool(name="pmat", bufs=4) as ppool, \
             tc.tile_pool(name="evict", bufs=3) as epool, \
             tc.tile_pool(name="hout", bufs=3) as hpool, \
             tc.tile_pool(name="psA", bufs=4, space="PSUM") as psA, \
             tc.tile_pool(name="psB", bufs=2, space="PSUM") as psB, \
             tc.tile_pool(name="dram", bufs=1, space="DRAM") as dpool:

            w_t = [cpool.tile([P, P], FP16, name=f"w{l}") for l in range(2)]
            b_t = [cpool.tile([1, P], FP16, name=f"b{l}") for l in range(2)]
            wld_t = [cpool.tile([P, P], FP32, name=f"wld{l}") for l in range(2)]
            bld_t = [cpool.tile([1, P], FP32, name=f"bld{l}") for l in range(2)]
            ones_t = cpool.tile([1, P], FP16)
            idx_t = cpool.tile([P, n16], mybir.dt.int16)
            partA = cpool.tile([P, SLOTS * P], FP16, name="partA")

            for l, (wi, bi) in enumerate([(w1_in, b1_in), (w2_in, b2_in)]):
                nc.sync.dma_start(out=wld_t[l][:], in_=wi[:])
                nc.sync.dma_start(out=bld_t[l][:], in_=bi[:])
                nc.vector.tensor_copy(out=w_t[l][:], in_=wld_t[l][:])
                nc.vector.tensor_copy(out=b_t[l][:], in_=bld_t[l][:])
            nc.vector.memset(ones_t[:], 1.0)
            nc.sync.dma_start(out=idx_t[:], in_=idx_in[:])

            h_shard = dpool.tile([SHARD, NFEAT], FP16, name="h_shard")
            h_fullA = dpool.tile([NFA, NFEAT], FP16, name="h_fullA")
            h_fullB = dpool.tile([NFB, NFEAT], FP16, name="h_fullB")

            def finish_slot(l, aggsrc, s, dst_ap, out_dt):
                """W matmul + bias + relu + writeback from fp16 agg lhsT."""
                h_ps = psB.tile([P, P], FP32, space="PSUM", name="hps", tag="hps")
                nc.tensor.matmul(
                    out=h_ps[:], lhsT=aggsrc, rhs=w_t[l][:],
                    start=True, stop=False,
                )
                nc.tensor.matmul(
                    out=h_ps[:], lhsT=ones_t[0:1, :], rhs=b_t[l][0:1, :],
                    start=False, stop=True,
                )
                h_sb = hpool.tile([P, P], out_dt, name="hout", tag=f"hout{l}")
                nc.scalar.activation(out=h_sb[:], in_=h_ps[:], func=relu)
                nc.sync.dma_start(out=dst_ap[s * P:(s + 1) * P, :], in_=h_sb[:])

            # ---------------- layer 1 (slot-major, no device gathers) ------
            for gi, g in enumerate(groups):
                slabs = []  # per q: (tile, seg_t0, npos of s offsets)
                for q in range(2):
                    t0, segn = seg[q][gi]
                    if segn == 0:
                        slabs.append((None, t0, 0))
                        continue
                    gb = gpool.tile([P, segn * P], FP16, name="gb", tag="g")
                    pg = ppool.tile([P, segn * P], FP16, name="pg", tag="p")
                    nc.sync.dma_start(out=gb[:], in_=xg1_in[:, t0 * P:(t0 + segn) * P])
                    nc.sync.dma_start(out=pg[:], in_=pmt_in[:, t0 * P:(t0 + segn) * P])
                    slabs.append(((gb, pg), t0, segn))
                posq = [0, 0]
                for s in g:
                    aggT = psA.tile([P, P], FP32, space="PSUM",
                                    name="aggT", tag="aggT")
                    work = []
                    for q in range(2):
                        for t in range(int(T_of[q][s])):
                            work.append((q, posq[q] + t))
                        posq[q] += int(T_of[q][s])
                    for k, (q, pos) in enumerate(work):
                        gb, pg = slabs[q][0]
                        nc.tensor.matmul(
                            out=aggT[:],
                            lhsT=gb[:, pos * P:(pos + 1) * P],
                            rhs=pg[:, pos * P:(pos + 1) * P],
                            start=(k == 0),
                            stop=(k == len(work) - 1),
                        )
                    aggT_sb = epool.tile([P, P], FP16, name="evict", tag="evict")
                    nc.scalar.copy(out=aggT_sb[:], in_=aggT[:])
                    finish_slot(0, aggT_sb[:], s, h_shard[:], FP16)
                if g[-1] == ASLOTS - 1:
                    nc.gpsimd.collective_compute(
                        "AllGather", mybir.AluOpType.bypass,
                        replica_groups=[list(range(NC))],
                        ins=[h_shard[0:SHA, :]], outs=[h_fullA[:]],
                    )

            # ---------------- layer 2 part A (overlaps nothing upstream of
            # AG_A; Tile overlaps these gathers with layer-1 B-slot compute)
            pm_slabs = {}
            agg_of = {}  # s -> partial psum evicted flag
            for phase in range(2):
                srcbuf = h_fullA if phase == 0 else h_fullB
                if phase == 1:
                    nc.gpsimd.collective_compute(
                        "AllGather", mybir.AluOpType.bypass,
                        replica_groups=[list(range(NC))],
                        ins=[h_shard[SHA:SHARD, :]], outs=[h_fullB[:]],
                    )
                for gi, g in enumerate(groups):
                    t0, segn = seg[phase][gi]
                    if segn == 0:
                        continue
                    gb = gpool.tile([P, segn * P], FP16, name="gb2",
                                    tag="g")
                    pg = ppool.tile([P, segn * P], FP16, name="pg2",
                                    tag="p")
                    nc.sync.dma_start(out=pg[:], in_=pmt_in[:, t0 * P:(t0 + segn) * P])
                    w0 = 0
                    while w0 < segn:
                        nt = min(CALL_TILES, segn - w0)
                        nidx = nt * P
                        gtid = t0 + w0
                        nc.gpsimd.dma_gather(
                            out_ap=gb[:, w0 * P:w0 * P + nidx].rearrange(
                                "p (t e) -> p t e", e=P
                            ),
                            in_ap=srcbuf[:],
                            idxs_ap=idx_t[:, gtid * 8:gtid * 8 + nidx // 16],
                            num_idxs=nidx,
                            num_idxs_reg=nidx,
                            elem_size=P,
                            single_packet=SINGLE_PACKET,
                        )
                        w0 += nt
                    pos = 0
                    for s in g:
                        ntot = int(T_of[phase][s])
                        if ntot == 0:
                            continue
                        aggT = psA.tile([P, P], FP32, space="PSUM",
                                        name="aggT2", tag="aggT")
                        for k in range(ntot):
                            nc.tensor.matmul(
                                out=aggT[:],
                                lhsT=gb[:, (pos + k) * P:(pos + k + 1) * P],
                                rhs=pg[:, (pos + k) * P:(pos + k + 1) * P],
                                start=(k == 0),
                                stop=(k == ntot - 1),
                            )
                        pos += ntot
                        if phase == 0:
                            nc.scalar.copy(
                                out=partA[:, s * P:(s + 1) * P], in_=aggT[:]
                            )
                            agg_of[s] = True
                        else:
                            agg_sb = epool.tile([P, P], FP16, name="evict2",
                                                tag="evict")
                            if agg_of.get(s):
                                nc.vector.tensor_tensor(
                                    out=agg_sb[:], in0=aggT[:],
                                    in1=partA[:, s * P:(s + 1) * P],
                                    op=mybir.AluOpType.add,
                                )
                            else:
                                nc.scalar.copy(out=agg_sb[:], in_=aggT[:])
                            finish_slot(1, agg_sb[:], s, out[:], FP32)
                # slots with no B tiles at all: finish from the A partial
                if phase == 1:
                    for g in groups:
                        for s in g:
                            if int(TB[s]) == 0 and agg_of.get(s):
                                finish_slot(
                                    1, partA[:, s * P:(s + 1) * P], s,
                                    out[:], FP32,
                                )

    nc.finalize()
    return nc


def kernel(x, edge_index, edge_weight, W1, b1, W2, b2):
    global last_run_results
    x = np.ascontiguousarray(np.asarray(x, dtype=np.float32))
    edge_index = np.asarray(edge_index)
    edge_weight = np.asarray(edge_weight, dtype=np.float32)

    block_at, sched, idx_np, pmt_np, rows_np = _prep(edge_index, edge_weight)
    n16 = idx_np[0].shape[1]
    nc = _build(sched, n16)

    x16 = x.astype(np.float16)
    ntiles = sched["ntiles"]
    in_maps = []
    for c in range(NC):
        # layer-1 gather on host: xg1[p, t*128+f] = x16[src[t*128+p], f]
        xg1 = np.ascontiguousarray(
            x16[rows_np[c].reshape(ntiles, P)]  # [t, p, f]
            .transpose(1, 0, 2).reshape(P, ntiles * P)
        )
        in_maps.append({
            "xg1": xg1,
            "W1": np.ascontiguousarray(W1, dtype=np.float32),
            "W2": np.ascontiguousarray(W2, dtype=np.float32),
            "b1": np.ascontiguousarray(b1, dtype=np.float32).reshape(1, NFEAT),
            "b2": np.ascontiguousarray(b2, dtype=np.float32).reshape(1, NFEAT),
            "idx": idx_np[c],
            "pmt": pmt_np[c],
        })

    trace = bool(int(os.environ.get("GCN_TRACE", "0")))
    res = run_bass_kernel_spmd(nc, in_maps, list(range(NC)), trace=trace)
    last_run_results = res

    full = np.zeros((NFULL, NFEAT), np.float32)
    for c in range(NC):
        shard = res.results[c]["out"]
        for s in range(SLOTS):
            b = int(block_at[c, s])
            full[b * P:(b + 1) * P] = shard[s * P:(s + 1) * P]
    return full[:N_NODES]


# revision 29
# speedup vs baseline: 1.1609x; 1.1609x over previous
"""Two-layer GCN encoder on 8 Trainium2 NeuronCores.

Strategy (dst-partitioned, matmul-based segment sum, fp16 internal):
  - Nodes are grouped into 392 blocks of 128; blocks are assigned to
    (core, slot) pairs balancing edge counts, 49 slots per core.
  - Every edge is owned by the core owning its dst block, so each core's
    aggregation for its blocks is complete: no all-reduce needed.
  - Per edge tile (128 edges): get the src rows into SBUF partitions
    (layer 1: host-pregathered xg1 slab loads; layer 2: dma_gather from
    the h pieces), and accumulate aggT[feat, node] += Xg.T @ P in PSUM,
    where P[e, n] = (dstcol==n)*w is HOST-PREBUILT (pmt input, shared by
    both layers) so no on-device P construction is needed.
  - Per block: h = relu(aggT.T @ W + b) via two matmuls (bias as a K=1
    matmul) and an ACT relu eviction (fp16 for layer 1, fp32 output for
    layer 2).

Overlap structure: slots are split at 25 into parts A/B.  h is
all-gathered in TWO pieces (AG_A after layer-1 slots 0-24, AG_B at the
end of layer 1).  Edges are partitioned by their SRC slot's part; layer 2
runs part-A gathers (reading h_fullA) concurrently with the tail of
layer-1 compute, then part-B.  Each piece's row space (25600 / 24576
rows) fits dma_gather's int16 indices, replacing the old lo/hi split.
Per-dst-slot aggregates accumulate part A into an fp16 partial, combined
with part B in a DVE add before the dense W matmul.

dma_gather is limited to 1024 indices/call (ring capacity: the device
hard-crashes above it) and costs ~8us/call of serial GpSimd time, which
is the main remaining bottleneck.
"""

import os

import numpy as np
from concourse import bacc, bass, mybir, tile
from concourse.bass_utils import run_bass_kernel_spmd

P = 128
N_NODES = 50000
N_EDGES = 800000
NFEAT = 128
NC = 8
SLOTS = 49                 # node blocks per core
NB = NC * SLOTS            # 392 blocks, 50176 padded rows
SHARD = SLOTS * P          # 6272 rows per core
NFULL = NB * P             # 50176
ASLOTS = 25                # slots in part A (rest in part B)
BSLOTS = SLOTS - ASLOTS
SHA = ASLOTS * P           # 3200 rows per core in piece A
SHB = BSLOTS * P           # 3072 rows per core in piece B
NFA = NC * SHA             # 25600 (< 32768: fits int16 gather idx)
NFB = NC * SHB             # 24576
GROUP = 5                  # dst slots per gather group
CALL_TILES = 8             # dma_gather device-crashes above 1024 idxs/call

SINGLE_PACKET = bool(int(os.environ.get("GCN_SP", "0")))

FP32 = mybir.dt.float32
FP16 = mybir.dt.float16

# Set by kernel() for test harness introspection (trace results etc.)
last_run_results = None


def _wrap16(flat):
    """dma_gather index layout: logical i -> [i % 16, i // 16], x8 replicated."""
    n16 = len(flat) // 16
    arr = np.asarray(flat, dtype=np.int16).reshape(n16, 16).T  # [16, n16]
    return np.tile(arr, (8, 1))  # [128, n16]


def _prep(edge_index, edge_weight):
    """Host-side sharding: block assignment, piece gather indices, pm tiles."""
    src = edge_index[0].astype(np.int64)
    dst = edge_index[1].astype(np.int64)
    w = edge_weight.astype(np.float32)

    blk = dst >> 7
    col = (dst & 127).astype(np.float32)

    cnt = np.bincount(blk, minlength=NB)
    order = np.argsort(-cnt, kind="stable")
    # Refine within slabs of 4 slots: re-sort by A-src edge count so each
    # slot's 8 blocks have similar A/B splits (reduces the shared
    # max-over-cores tile schedule).  Partness depends on the assignment,
    # so approximate with a first-pass assignment by total count.
    slot_of0 = np.empty(NB, np.int64)
    ba0 = order.reshape(SLOTS, NC).T
    for c0 in range(NC):
        for s0 in range(SLOTS):
            slot_of0[ba0[c0, s0]] = s0
    sblk0 = src >> 7
    a0 = slot_of0[sblk0] < ASLOTS
    a_cnt = np.bincount(blk[a0], minlength=NB)
    order2 = order.copy()
    for a in range(0, NB, 4 * NC):
        slab = order2[a:a + 4 * NC]
        order2[a:a + 4 * NC] = slab[np.argsort(-a_cnt[slab], kind="stable")]
    block_at = order2.reshape(SLOTS, NC).T          # [core, slot] -> block
    core_of = np.empty(NB, np.int64)
    slot_of = np.empty(NB, np.int64)
    for c in range(NC):
        for s in range(SLOTS):
            core_of[block_at[c, s]] = c
            slot_of[block_at[c, s]] = s

    eorder = np.argsort(blk, kind="stable")
    estart = np.zeros(NB + 1, np.int64)
    np.cumsum(cnt, out=estart[1:])

    # per-edge src part + piece-space gather row
    sblk = src >> 7
    s_core = core_of[sblk]
    s_slot = slot_of[sblk]
    is_a = s_slot < ASLOTS
    vq = np.where(
        is_a,
        s_core * SHA + s_slot * P + (src & 127),
        s_core * SHB + (s_slot - ASLOTS) * P + (src & 127),
    )

    groups = [list(range(g, min(g + GROUP, SLOTS))) for g in range(0, SLOTS, GROUP)]
    NG = len(groups)

    # per (core, slot): A/B edge id lists + shared tile schedule
    ids_cs = [[None] * SLOTS for _ in range(NC)]
    TA = np.zeros(SLOTS, np.int64)
    TB = np.zeros(SLOTS, np.int64)
    for c in range(NC):
        for s in range(SLOTS):
            b = block_at[c, s]
            ids = eorder[estart[b]:estart[b + 1]]
            m = is_a[ids]
            ea, eb = ids[m], ids[~m]
            ids_cs[c][s] = (ea, eb)
            TA[s] = max(TA[s], (len(ea) + P - 1) // P)
            TB[s] = max(TB[s], (len(eb) + P - 1) // P)

    # Tile enumeration: part-major: for q in (A, B): for g: for s in g.
    T_of = [TA, TB]
    seg = []          # [q][g] -> (tid0, ntiles_in_segment)
    tid0 = 0
    for q in range(2):
        segq = []
        for g in groups:
            n = int(sum(T_of[q][s] for s in g))
            segq.append((tid0, n))
            tid0 += n
        seg.append(segq)
    ntiles = tid0

    # gather calls for layer 2: per (q, g) segment, windows of <=8 tiles
    calls = []  # (q, gi, t0_in_seg, ntiles)
    for q in range(2):
        for gi in range(NG):
            _, segn = seg[q][gi]
            t0 = 0
            while t0 < segn:
                nt = min(CALL_TILES, segn - t0)
                calls.append((q, gi, t0, nt))
                t0 += nt

    sched = {"TA": TA, "TB": TB, "groups": groups, "seg": seg,
             "ntiles": ntiles, "calls": calls}

    idx_np = []
    pmt_np = []
    colw_np = []
    rows_np = []
    ncols = np.arange(P, dtype=np.float32)
    for c in range(NC):
        flat_idx = []
        flat_rows = []
        pmt = np.zeros((P, ntiles * P), np.float16)
        colw = np.zeros((P, 2 * ntiles), np.float32)
        tid = 0
        for q in range(2):
            for g in groups:
                for s in g:
                    ea, eb = ids_cs[c][s]
                    ids = ea if q == 0 else eb
                    n = int(T_of[q][s]) * P
                    iv = np.zeros(n, np.int64)
                    rv = np.zeros(n, np.int64)
                    cv = np.full(n, -1.0, np.float32)
                    wv = np.zeros(n, np.float32)
                    iv[:len(ids)] = vq[ids]
                    rv[:len(ids)] = src[ids]
                    cv[:len(ids)] = col[ids]
                    wv[:len(ids)] = w[ids]
                    flat_idx.append(iv)
                    flat_rows.append(rv)
                    for t in range(int(T_of[q][s])):
                        pm = (cv[t * P:(t + 1) * P, None] == ncols[None, :])
                        pm = pm * wv[t * P:(t + 1) * P, None]
                        pmt[:, tid * P:(tid + 1) * P] = pm.astype(np.float16)
                        colw[:, 2 * tid] = cv[t * P:(t + 1) * P]
                        colw[:, 2 * tid + 1] = wv[t * P:(t + 1) * P]
                        tid += 1
        idx_np.append(_wrap16(np.concatenate(flat_idx)))
        rows_np.append(np.concatenate(flat_rows))
        pmt_np.append(pmt)
        colw_np.append(colw)

    return block_at, sched, idx_np, pmt_np, colw_np, rows_np


def _build(sched, n16):
    """Build the SPMD bass program. Returns finalized nc."""
    nc = bacc.Bacc(num_devices=NC)

    ntiles = sched["ntiles"]
    # Layer-1 gather done on the HOST: xg1[p, t*128 + f] = x16[src[t*128+p], f]
    # in exactly the SBUF layout dma_gather would produce.
    xg1_in = nc.declare_dram_parameter("xg1", [P, ntiles * P], FP16, isOutput=False)
    # host-built P tiles: pmt[e, tid*128+n] = (dstcol==n)*w, both layers
    pmt_in = nc.declare_dram_parameter("pmt", [P, ntiles * P], FP16, isOutput=False)
    w1_in = nc.declare_dram_parameter("W1", [NFEAT, NFEAT], FP32, isOutput=False)
    w2_in = nc.declare_dram_parameter("W2", [NFEAT, NFEAT], FP32, isOutput=False)
    b1_in = nc.declare_dram_parameter("b1", [1, NFEAT], FP32, isOutput=False)
    b2_in = nc.declare_dram_parameter("b2", [1, NFEAT], FP32, isOutput=False)
    idx_in = nc.declare_dram_parameter("idx", [P, n16], mybir.dt.int16, isOutput=False)
    iota_in = nc.declare_dram_parameter("iota", [P, P], FP16, isOutput=False)
    colw_in = nc.declare_dram_parameter("colw", [P, 2 * ntiles], FP32, isOutput=False)
    out = nc.declare_dram_parameter("out", [SHARD, NFEAT], FP32, isOutput=True)

    relu = mybir.ActivationFunctionType.Relu
    TA, TB = sched["TA"], sched["TB"]
    T_of = [TA, TB]
    groups = sched["groups"]
    seg = sched["seg"]

    with tile.TileContext(nc) as tc:
        with tc.tile_pool(name="const", bufs=1) as cpool, \
             tc.tile_pool(name="gbuf", bufs=4) as gpool, \
             tc.tile_pool(name="pmat", bufs=4) as ppool, \
             tc.tile_pool(name="g2buf", bufs=3) as g2pool, \
             tc.tile_pool(name="pm2", bufs=16) as pm2pool, \
             tc.tile_pool(name="evict", bufs=3) as epool, \
             tc.tile_pool(name="hout", bufs=3) as hpool, \
             tc.tile_pool(name="psA", bufs=4, space="PSUM") as psA, \
             tc.tile_pool(name="psB", bufs=2, space="PSUM") as psB, \
             tc.tile_pool(name="dram", bufs=1, space="DRAM") as dpool:

            w_t = [cpool.tile([P, P], FP16, name=f"w{l}") for l in range(2)]
            b_t = [cpool.tile([1, P], FP16, name=f"b{l}") for l in range(2)]
            wld_t = [cpool.tile([P, P], FP32, name=f"wld{l}") for l in range(2)]
            bld_t = [cpool.tile([1, P], FP32, name=f"bld{l}") for l in range(2)]
            ones_t = cpool.tile([1, P], FP16)
            idx_t = cpool.tile([P, n16], mybir.dt.int16)
            iota_t = cpool.tile([P, P], FP16)
            colw_t = cpool.tile([P, 2 * ntiles], FP32)
            partA = cpool.tile([P, SLOTS * P], FP16, name="partA")

            for l, (wi, bi) in enumerate([(w1_in, b1_in), (w2_in, b2_in)]):
                nc.sync.dma_start(out=wld_t[l][:], in_=wi[:])
                nc.sync.dma_start(out=bld_t[l][:], in_=bi[:])
                nc.vector.tensor_copy(out=w_t[l][:], in_=wld_t[l][:])
                nc.vector.tensor_copy(out=b_t[l][:], in_=bld_t[l][:])
            nc.vector.memset(ones_t[:], 1.0)
            nc.sync.dma_start(out=idx_t[:], in_=idx_in[:])
            nc.sync.dma_start(out=iota_t[:], in_=iota_in[:])
            nc.sync.dma_start(out=colw_t[:], in_=colw_in[:])

            h_shard = dpool.tile([SHARD, NFEAT], FP16, name="h_shard")
            h_fullA = dpool.tile([NFA, NFEAT], FP16, name="h_fullA")
            h_fullB = dpool.tile([NFB, NFEAT], FP16, name="h_fullB")

            def finish_slot(l, aggsrc, s, dst_ap, out_dt):
                """W matmul + bias + relu + writeback from fp16 agg lhsT."""
                h_ps = psB.tile([P, P], FP32, space="PSUM", name="hps", tag="hps")
                nc.tensor.matmul(
                    out=h_ps[:], lhsT=aggsrc, rhs=w_t[l][:],
                    start=True, stop=False,
                )
                nc.tensor.matmul(
                    out=h_ps[:], lhsT=ones_t[0:1, :], rhs=b_t[l][0:1, :],
                    start=False, stop=True,
                )
                h_sb = hpool.tile([P, P], out_dt, name="hout", tag=f"hout{l}")
                nc.scalar.activation(out=h_sb[:], in_=h_ps[:], func=relu)
                nc.sync.dma_start(out=dst_ap[s * P:(s + 1) * P, :], in_=h_sb[:])

            # ---------------- layer 1 (slot-major, no device gathers) ------
            for gi, g in enumerate(groups):
                slabs = []  # per q: (tile, seg_t0, npos of s offsets)
                for q in range(2):
                    t0, segn = seg[q][gi]
                    if segn == 0:
                        slabs.append((None, t0, 0))
                        continue
                    gb = gpool.tile([P, segn * P], FP16, name="gb", tag="g")
                    pg = ppool.tile([P, segn * P], FP16, name="pg", tag="p")
                    nc.sync.dma_start(out=gb[:], in_=xg1_in[:, t0 * P:(t0 + segn) * P])
                    nc.sync.dma_start(out=pg[:], in_=pmt_in[:, t0 * P:(t0 + segn) * P])
                    slabs.append(((gb, pg), t0, segn))
                posq = [0, 0]
                for s in g:
                    aggT = psA.tile([P, P], FP32, space="PSUM",
                                    name="aggT", tag="aggT")
                    work = []
                    for q in range(2):
                        for t in range(int(T_of[q][s])):
                            work.append((q, posq[q] + t))
                        posq[q] += int(T_of[q][s])
                    for k, (q, pos) in enumerate(work):
                        gb, pg = slabs[q][0]
                        nc.tensor.matmul(
                            out=aggT[:],
                            lhsT=gb[:, pos * P:(pos + 1) * P],
                            rhs=pg[:, pos * P:(pos + 1) * P],
                            start=(k == 0),
                            stop=(k == len(work) - 1),
                        )
                    aggT_sb = epool.tile([P, P], FP16, name="evict", tag="evict")
                    nc.scalar.copy(out=aggT_sb[:], in_=aggT[:])
                    finish_slot(0, aggT_sb[:], s, h_shard[:], FP16)
                if g[-1] == ASLOTS - 1:
                    nc.gpsimd.collective_compute(
                        "AllGather", mybir.AluOpType.bypass,
                        replica_groups=[list(range(NC))],
                        ins=[h_shard[0:SHA, :]], outs=[h_fullA[:]],
                    )

            # ---------------- layer 2 part A (overlaps nothing upstream of
            # AG_A; Tile overlaps these gathers with layer-1 B-slot compute)
            pm_slabs = {}
            agg_of = {}  # s -> partial psum evicted flag
            for phase in range(2):
                srcbuf = h_fullA if phase == 0 else h_fullB
                if phase == 1:
                    nc.gpsimd.collective_compute(
                        "AllGather", mybir.AluOpType.bypass,
                        replica_groups=[list(range(NC))],
                        ins=[h_shard[SHA:SHARD, :]], outs=[h_fullB[:]],
                    )
                for gi, g in enumerate(groups):
                    t0, segn = seg[phase][gi]
                    if segn == 0:
                        continue
                    gb = g2pool.tile([P, segn * P], FP16, name="gb2",
                                     tag="g2")
                    w0 = 0
                    while w0 < segn:
                        nt = min(CALL_TILES, segn - w0)
                        nidx = nt * P
                        gtid = t0 + w0
                        nc.gpsimd.dma_gather(
                            out_ap=gb[:, w0 * P:w0 * P + nidx].rearrange(
                                "p (t e) -> p t e", e=P
                            ),
                            in_ap=srcbuf[:],
                            idxs_ap=idx_t[:, gtid * 8:gtid * 8 + nidx // 16],
                            num_idxs=nidx,
                            num_idxs_reg=nidx,
                            elem_size=P,
                            single_packet=SINGLE_PACKET,
                        )
                        w0 += nt
                    pos = 0
                    for s in g:
                        ntot = int(T_of[phase][s])
                        if ntot == 0:
                            continue
                        aggT = psA.tile([P, P], FP32, space="PSUM",
                                        name="aggT2", tag="aggT")
                        for k in range(ntot):
                            tid = t0 + pos + k
                            pm2 = pm2pool.tile([P, P], FP16, name="pm2",
                                               tag="pm2")
                            nc.vector.tensor_scalar(
                                out=pm2[:],
                                in0=iota_t[:],
                                scalar1=colw_t[:, 2 * tid:2 * tid + 1],
                                scalar2=colw_t[:, 2 * tid + 1:2 * tid + 2],
                                op0=mybir.AluOpType.is_equal,
                                op1=mybir.AluOpType.mult,
                            )
                            nc.tensor.matmul(
                                out=aggT[:],
                                lhsT=gb[:, (pos + k) * P:(pos + k + 1) * P],
                                rhs=pm2[:],
                                start=(k == 0),
                                stop=(k == ntot - 1),
                            )
                        pos += ntot
                        if phase == 0:
                            nc.scalar.copy(
                                out=partA[:, s * P:(s + 1) * P], in_=aggT[:]
                            )
                            agg_of[s] = True
                        else:
                            agg_sb = epool.tile([P, P], FP16, name="evict2",
                                                tag="evict")
                            if agg_of.get(s):
                                nc.vector.tensor_tensor(
                                    out=agg_sb[:], in0=aggT[:],
                                    in1=partA[:, s * P:(s + 1) * P],
                                    op=mybir.AluOpType.add,
                                )
                            else:
                                nc.scalar.copy(out=agg_sb[:], in_=aggT[:])
                            finish_slot(1, agg_sb[:], s, out[:], FP32)
                # slots with no B tiles at all: finish from the A partial
                if phase == 1:
                    for g in groups:
                        for s in g:
                            if int(TB[s]) == 0 and agg_of.get(s):
                                finish_slot(
                                    1, partA[:, s * P:(s + 1) * P], s,
                                    out[:], FP32,
                                )

    nc.finalize()
    return nc


def kernel(x, edge_index, edge_weight, W1, b1, W2, b2):
    global last_run_results
    x = np.ascontiguousarray(np.asarray(x, dtype=np.float32))
    edge_index = np.asarray(edge_index)
    edge_weight = np.asarray(edge_weight, dtype=np.float32)

    block_at, sched, idx_np, pmt_np, colw_np, rows_np = _prep(edge_index, edge_weight)
    n16 = idx_np[0].shape[1]
    nc = _build(sched, n16)

    x16 = x.astype(np.float16)
    iota_np = np.broadcast_to(np.arange(P, dtype=np.float16), (P, P)).copy()
    ntiles = sched["ntiles"]
    in_maps = []
    for c in range(NC):
        # layer-1 gather on host: xg1[p, t*128+f] = x16[src[t*128+p], f]
        xg1 = np.ascontiguousarray(
            x16[rows_np[c].reshape(ntiles, P)]  # [t, p, f]
            .transpose(1, 0, 2).reshape(P, ntiles * P)
        )
        in_maps.append({
            "xg1": xg1,
            "W1": np.ascontiguousarray(W1, dtype=np.float32),
            "W2": np.ascontiguousarray(W2, dtype=np.float32),
            "b1": np.ascontiguousarray(b1, dtype=np.float32).reshape(1, NFEAT),
            "b2": np.ascontiguousarray(b2, dtype=np.float32).reshape(1, NFEAT),
            "idx": idx_np[c],
            "pmt": pmt_np[c],
            "iota": iota_np,
            "colw": colw_np[c],
        })

    trace = bool(int(os.environ.get("GCN_TRACE", "0")))
    res = run_bass_kernel_spmd(nc, in_maps, list(range(NC)), trace=trace)
    last_run_results = res

    full = np.zeros((NFULL, NFEAT), np.float32)
    for c in range(NC):
        shard = res.results[c]["out"]
        for s in range(SLOTS):
            b = int(block_at[c, s])
            full[b * P:(b + 1) * P] = shard[s * P:(s + 1) * P]
    return full[:N_NODES]
